# revision 10
# baseline (speedup 1.0000x reference)
"""Trainium2 Bass kernel: single-head causal attention head layer.

Reference computation (per batch b):
    q = x[b] @ Wq; k = x[b] @ Wk; v = x[b] @ Wv        # [S, H], H=64
    w = softmax_causal(q @ k.T * E**-0.5)              # [S, S]
    out[b] = w @ v                                     # [S, H]

Shapes: x (8, 2048, 1024) f32, Wq/Wk/Wv (1024, 64) f32 -> out (8, 2048, 64) f32.

Sharding: data-parallel over batch, one batch per NeuronCore (8 cores).

Device algorithm (per core), all matmuls bf16 with fp32 PSUM accumulation:
  1. Projections pipelined against the x^T DMA stream (per 128-row e-tile):
     [Wq|Wk] stationary -> qk^T psum [128, 2048] (rows 0:64 q^T, 64:128 k^T);
     Wv stationary -> v^T psum [64, 2048].
  2. k^T copied to rows 0:64 of a [128, 2048] tile whose rows 64:128 are
     zero. Scores can then contract over K=128 against the full qk tile
     (zero weights kill the k^T rows): full PE array activity, which keeps
     the HAM clock gate at 2.4 GHz. v^T is PE-transposed into 16 tiles
     v_aug [128, 128]: cols 0:64 = v, col 64 = ones (row sums of P fall out
     of the AV matmul for free), cols 65:128 = zero (pads M to 128).
  3. Scores transposed: S^T[j,i] = k_j . q_i, keys on partitions, so the
     softmax denominator is a partition-dim sum -> folded into step 5.
  4. exp on ScalarE with scale=E**-0.5, 1024-wide chunks. No
     max-subtraction: scaled scores are N(0, 0.0625), exp is safe. Causal
     masking: block skip + multiplicative 0/1 bf16 mask on diagonal chunks.
  5. O^T_aug[h,i] = sum_j v_aug[j,h] P^T[j,i] into four 512-col psum
     region tiles; row 64 = denominators. When a region's last j arrives,
     it is finalized immediately (overlaps the remaining attention):
  6. PE-transpose 128-col slices -> [128, 128]; reciprocal of col 64;
     scale cols 0:64; DMA out fp32.
"""

import numpy as np
import ml_dtypes

BATCH = 8
SEQ = 2048
EMBED = 1024
HEAD = 64
N_CORES = 8
SCALE = float(EMBED) ** -0.5  # 0.03125

ST = SEQ // 128  # 16 seq tiles
ET = EMBED // 128  # 8 embed tiles

_CACHE = {}


def _build_program():
    import concourse.mybir as mybir
    from concourse import bacc
    from concourse.tile import TileContext

    f32 = mybir.dt.float32
    bf16 = mybir.dt.bfloat16
    EXP = mybir.ActivationFunctionType.Exp

    nc = bacc.Bacc("TRN2", target_bir_lowering=False, debug=False,
                   num_devices=N_CORES)

    xT = nc.declare_dram_parameter("xT", [EMBED, SEQ], bf16, isOutput=False)
    wqk = nc.declare_dram_parameter("wqk", [128, ET, 128], bf16, isOutput=False)
    wv = nc.declare_dram_parameter("wv", [128, ET, HEAD], bf16, isOutput=False)
    ident = nc.declare_dram_parameter("ident", [128, 128], f32, isOutput=False)
    ident64 = nc.declare_dram_parameter("ident64", [HEAD, HEAD], bf16,
                                        isOutput=False)
    out = nc.declare_dram_parameter("out", [SEQ, HEAD], f32, isOutput=True)

    with TileContext(nc) as tc:
        with (
            tc.tile_pool(name="persist", bufs=1) as persist,
            tc.tile_pool(name="xtp", bufs=1) as xtp,
            tc.tile_pool(name="vtiles", bufs=1) as vtiles,
            tc.tile_pool(name="psb", bufs=4) as psb,
            tc.tile_pool(name="osb", bufs=4) as osb,
            tc.tile_pool(name="rsb", bufs=4) as rsb,
        ):
            # ---- weights/constants; two issue streams (sync + gpsimd) ----
            wqk_sb = persist.tile([128, ET, 128], bf16)
            nc.sync.dma_start(out=wqk_sb[:], in_=wqk[:])
            wv_sb = persist.tile([128, ET, HEAD], bf16)
            nc.gpsimd.dma_start(out=wv_sb[:], in_=wv[:])
            id64_sb = persist.tile([HEAD, HEAD], bf16)
            nc.gpsimd.dma_start(out=id64_sb[:], in_=ident64[:])

            # preload the exp table set so ACT_TABLE_LOAD overlaps the DMAs
            warm_sb = persist.tile([128, 1], f32)
            nc.vector.memset(warm_sb[:], 0.0)
            nc.scalar.activation(warm_sb[:], warm_sb[:], EXP, scale=1.0)

            qk_sbs, kt2_sbs, vt_sbs = [], [], []
            for c in range(4):
                qk_sbs.append(persist.tile([128, 512], bf16, name=f"qk{c}"))
                kt2_sbs.append(persist.tile([128, 512], bf16, name=f"kt2{c}"))
                nc.vector.memset(kt2_sbs[c][64:128, :], 0.0)
                vt_sbs.append(persist.tile([64, 512], bf16, name=f"vt{c}"))
            v_sbs = []
            for s in range(ST):
                v_sbs.append(vtiles.tile([128, 128], bf16,
                                         name=f"v{s}", tag=f"v{s}"))
            ot_sb = persist.tile([128, SEQ], f32)

            # ---- Phase B: projections, pipelined against the xT DMA ----
            with tc.tile_pool(name="ps_b", bufs=1, space="PSUM") as ps_b:
                qk_ps, vt_ps = [], []
                for c in range(4):
                    qk_ps.append(ps_b.tile([128, 512], f32, name=f"qkps{c}"))
                    vt_ps.append(ps_b.tile([64, 512], f32, name=f"vtps{c}"))
                for e in range(ET):
                    xt_e = xtp.tile([128, SEQ], bf16, name=f"xt{e}",
                                    tag=f"xt{e}")
                    eng = nc.sync if e % 2 == 0 else nc.gpsimd
                    eng.dma_start(out=xt_e[:],
                                  in_=xT[128 * e:128 * (e + 1), :])
                    for c in range(SEQ // 512):
                        nc.tensor.matmul(
                            qk_ps[c][:],
                            lhsT=wqk_sb[:, e, :],
                            rhs=xt_e[:, 512 * c:512 * (c + 1)],
                            start=(e == 0), stop=(e == ET - 1),
                        )
                    for c in range(SEQ // 512):
                        nc.tensor.matmul(
                            vt_ps[c][:],
                            lhsT=wv_sb[:, e, :],
                            rhs=xt_e[:, 512 * c:512 * (c + 1)],
                            start=(e == 0), stop=(e == ET - 1),
                        )
                id_sb = persist.tile([128, 128], f32)
                nc.gpsimd.dma_start(out=id_sb[:], in_=ident[:])

                for c in range(SEQ // 512):
                    nc.vector.tensor_copy(qk_sbs[c][:], qk_ps[c][:])
                    # shift kT (partitions 64:128) down to base partition 0
                    nc.sync.dma_start(out=kt2_sbs[c][0:64, :],
                                      in_=qk_sbs[c][64:128, :])
                    nc.vector.tensor_copy(vt_sbs[c][:], vt_ps[c][:])

            # ---- v layout fix: PE-transpose v^T 128-col slices ----
            with tc.tile_pool(name="ps_vt", bufs=2, space="PSUM") as ps_vt:
                for s in range(ST):
                    v_ps = ps_vt.tile([128, HEAD], bf16, tag="vps")
                    nc.tensor.transpose(
                        v_ps[:],
                        vt_sbs[s // 4][:, 128 * (s % 4):128 * (s % 4 + 1)],
                        id64_sb[:])
                    nc.vector.memset(v_sbs[s][:, HEAD:HEAD + 1], 1.0)
                    nc.vector.memset(v_sbs[s][:, HEAD + 1:128], 0.0)
                    nc.vector.tensor_copy(v_sbs[s][:, 0:HEAD], v_ps[:])

            # ---- Phase D: attention, with per-region finalization ----
            with (
                tc.tile_pool(name="ps_o", bufs=1, space="PSUM") as ps_o,
                tc.tile_pool(name="ps_s", bufs=2, space="PSUM") as ps_s,
            ):
                o_regs = []
                for g in range(4):
                    o_regs.append(ps_o.tile([128, 512], f32,
                                            name=f"oreg{g}", tag=f"oreg{g}"))

                def finalize_region(g):
                    # region g covers queries [512g, 512(g+1))
                    nc.vector.tensor_copy(ot_sb[:, 512 * g:512 * (g + 1)],
                                          o_regs[g][:])
                    for ss in range(4):
                        s = 4 * g + ss
                        t_ps = ps_s.tile([128, 128], f32, tag="sps")
                        nc.tensor.transpose(
                            t_ps[:], ot_sb[:, 128 * s:128 * (s + 1)],
                            id_sb[:])
                        recip = rsb.tile([128, 1], f32, tag="recip")
                        nc.vector.reciprocal(recip[:],
                                             t_ps[:, HEAD:HEAD + 1])
                        o_sb = osb.tile([128, HEAD], f32, tag="osb")
                        nc.vector.tensor_scalar_mul(o_sb[:], t_ps[:, 0:HEAD],
                                                    recip[:])
                        nc.gpsimd.dma_start(
                            out=out[128 * s:128 * (s + 1), :], in_=o_sb[:])

                for j in range(ST):
                    c0 = j // 8
                    lo = 128 * (j % 8)
                    for cc in range(c0, 2):
                        klo = lo if cc == c0 else 0
                        base = 1024 * cc
                        s_ps = ps_s.tile([128, 1024], f32, tag="sps")
                        kT = kt2_sbs[j // 4][:, 128 * (j % 4):128 * (j % 4 + 1)]
                        for h in (0, 512):
                            a = max(klo, h)
                            if a < h + 512:
                                nc.tensor.matmul(
                                    s_ps[:, a:h + 512],
                                    lhsT=kT,
                                    rhs=qk_sbs[2 * cc + h // 512][
                                        :, a - h:a - h + (h + 512 - a)],
                                    start=True, stop=True,
                                )
                        p_sb = psb.tile([128, 1024], bf16, tag="psb")
                        nc.scalar.activation(p_sb[:, klo:1024],
                                             s_ps[:, klo:1024],
                                             EXP, scale=SCALE)
                        if cc == c0:
                            # causal mask, in place on the idle GpSimd
                            # engine: keep where (1024*cc + y) >= 128*j + x
                            nc.gpsimd.affine_select(
                                out=p_sb[:, 0:1024], in_=p_sb[:, 0:1024],
                                compare_op=mybir.AluOpType.is_ge,
                                fill=0.0,
                                base=1024 * cc - 128 * j,
                                pattern=[[1, 1024]],
                                channel_multiplier=-1,
                            )
                        for hh in (0, 1):
                            g = 2 * cc + hh
                            if j > 4 * g + 3:
                                continue  # fully above causal diagonal
                            nc.tensor.matmul(
                                o_regs[g][:],
                                lhsT=v_sbs[j][:],
                                rhs=p_sb[:, 512 * hh:512 * (hh + 1)],
                                start=(j == 0), stop=(j == 4 * g + 3),
                            )
                            if j == 4 * g + 3:
                                finalize_region(g)

    nc.compile()
    return nc


def _get_program():
    if "nc" not in _CACHE:
        _CACHE["nc"] = _build_program()
    return _CACHE["nc"]


def _host_inputs(x, Wq, Wk, Wv):
    bf16 = ml_dtypes.bfloat16
    # x^T per batch: [E, S] contiguous, bf16
    xT = np.ascontiguousarray(x.transpose(0, 2, 1)).astype(bf16)
    # [Wq | Wk] -> [128, ET, 128] (partition = embed % 128)
    wqk = np.concatenate([Wq, Wk], axis=1).astype(bf16)  # [E, 128]
    wqk = np.ascontiguousarray(
        wqk.reshape(ET, 128, 128).transpose(1, 0, 2))  # [128, ET, 128]
    wv = np.ascontiguousarray(
        Wv.astype(bf16).reshape(ET, 128, HEAD).transpose(1, 0, 2))
    ident = np.eye(128, dtype=np.float32)
    ident64 = np.eye(HEAD, dtype=bf16)
    return xT, wqk, wv, ident, ident64


def kernel(x, Wq, Wk, Wv):
    from concourse.bass_utils import run_bass_kernel_spmd

    nc = _get_program()
    xT, wqk, wv, ident, ident64 = _host_inputs(x, Wq, Wk, Wv)
    in_maps = [
        {"xT": xT[b], "wqk": wqk, "wv": wv,
         "ident": ident, "ident64": ident64}
        for b in range(BATCH)
    ]
    res = run_bass_kernel_spmd(nc, in_maps, list(range(N_CORES)))
    out = np.stack([np.asarray(res.results[b]["out"]) for b in range(BATCH)])
    return out.astype(np.float32)


# revision 11
# speedup vs baseline: 1.0411x; 1.0411x over previous
"""Trainium2 Bass kernel: single-head causal attention head layer.

Reference computation (per batch b):
    q = x[b] @ Wq; k = x[b] @ Wk; v = x[b] @ Wv        # [S, H], H=64
    w = softmax_causal(q @ k.T * E**-0.5)              # [S, S]
    out[b] = w @ v                                     # [S, H]

Shapes: x (8, 2048, 1024) f32, Wq/Wk/Wv (1024, 64) f32 -> out (8, 2048, 64) f32.

Sharding: data-parallel over batch, one batch per NeuronCore (8 cores).

Device algorithm (per core), all matmuls bf16 with fp32 PSUM accumulation:
  1. Projections pipelined against the x^T DMA stream (per 128-row e-tile):
     [Wq|Wk] stationary -> qk^T psum [128, 2048] (rows 0:64 q^T, 64:128 k^T);
     Wv stationary -> v^T psum [64, 2048].
  2. k^T copied to rows 0:64 of a [128, 2048] tile whose rows 64:128 are
     zero. Scores can then contract over K=128 against the full qk tile
     (zero weights kill the k^T rows): full PE array activity, which keeps
     the HAM clock gate at 2.4 GHz. v^T is PE-transposed into 16 tiles
     v_aug [128, 128]: cols 0:64 = v, col 64 = ones (row sums of P fall out
     of the AV matmul for free), cols 65:128 = zero (pads M to 128).
  3. Scores transposed: S^T[j,i] = k_j . q_i, keys on partitions, so the
     softmax denominator is a partition-dim sum -> folded into step 5.
  4. exp on ScalarE with scale=E**-0.5, 1024-wide chunks. No
     max-subtraction: scaled scores are N(0, 0.0625), exp is safe. Causal
     masking: block skip + multiplicative 0/1 bf16 mask on diagonal chunks.
  5. O^T_aug[h,i] = sum_j v_aug[j,h] P^T[j,i] into four 512-col psum
     region tiles; row 64 = denominators. When a region's last j arrives,
     it is finalized immediately (overlaps the remaining attention):
  6. PE-transpose 128-col slices -> [128, 128]; reciprocal of col 64;
     scale cols 0:64; DMA out fp32.
"""

import numpy as np
import ml_dtypes

BATCH = 8
SEQ = 2048
EMBED = 1024
HEAD = 64
N_CORES = 8
SCALE = float(EMBED) ** -0.5  # 0.03125

ST = SEQ // 128  # 16 seq tiles
ET = EMBED // 128  # 8 embed tiles

_CACHE = {}


def _build_program():
    import concourse.mybir as mybir
    from concourse import bacc
    from concourse.tile import TileContext

    f32 = mybir.dt.float32
    bf16 = mybir.dt.bfloat16
    EXP = mybir.ActivationFunctionType.Exp

    nc = bacc.Bacc("TRN2", target_bir_lowering=False, debug=False,
                   num_devices=N_CORES)

    xT = nc.declare_dram_parameter("xT", [EMBED, SEQ], bf16, isOutput=False)
    wqk = nc.declare_dram_parameter("wqk", [128, ET, 128], bf16, isOutput=False)
    wv = nc.declare_dram_parameter("wv", [128, ET, HEAD], bf16, isOutput=False)
    mask128 = nc.declare_dram_parameter("mask128", [128, 128], bf16,
                                        isOutput=False)
    ident = nc.declare_dram_parameter("ident", [128, 128], f32, isOutput=False)
    ident64 = nc.declare_dram_parameter("ident64", [HEAD, HEAD], bf16,
                                        isOutput=False)
    out = nc.declare_dram_parameter("out", [SEQ, HEAD], f32, isOutput=True)

    with TileContext(nc) as tc:
        with (
            tc.tile_pool(name="persist", bufs=1) as persist,
            tc.tile_pool(name="xtp", bufs=1) as xtp,
            tc.tile_pool(name="vtiles", bufs=1) as vtiles,
            tc.tile_pool(name="psb", bufs=6) as psb,
            tc.tile_pool(name="osb", bufs=4) as osb,
            tc.tile_pool(name="rsb", bufs=4) as rsb,
        ):
            # ---- weights/constants; two issue streams (sync + gpsimd) ----
            wqk_sb = persist.tile([128, ET, 128], bf16)
            nc.sync.dma_start(out=wqk_sb[:], in_=wqk[:])
            wv_sb = persist.tile([128, ET, HEAD], bf16)
            nc.gpsimd.dma_start(out=wv_sb[:], in_=wv[:])
            id64_sb = persist.tile([HEAD, HEAD], bf16)
            nc.gpsimd.dma_start(out=id64_sb[:], in_=ident64[:])
            mask_sb = persist.tile([128, 128], bf16)
            nc.gpsimd.dma_start(out=mask_sb[:], in_=mask128[:])

            # preload the exp table set so ACT_TABLE_LOAD overlaps the DMAs
            warm_sb = persist.tile([128, 1], f32)
            nc.vector.memset(warm_sb[:], 0.0)
            nc.scalar.activation(warm_sb[:], warm_sb[:], EXP, scale=1.0)

            qk_sbs, kt2_sbs, vt_sbs = [], [], []
            for c in range(4):
                qk_sbs.append(persist.tile([128, 512], bf16, name=f"qk{c}"))
                kt2_sbs.append(persist.tile([128, 512], bf16, name=f"kt2{c}"))
                nc.vector.memset(kt2_sbs[c][64:128, :], 0.0)
                vt_sbs.append(persist.tile([64, 512], bf16, name=f"vt{c}"))
            v_sbs = []
            for s in range(ST):
                v_sbs.append(vtiles.tile([128, 128], bf16,
                                         name=f"v{s}", tag=f"v{s}"))
            ot_sb = persist.tile([128, SEQ], f32)

            # ---- Phase B: projections, pipelined against the xT DMA ----
            with tc.tile_pool(name="ps_b", bufs=1, space="PSUM") as ps_b:
                qk_ps, vt_ps = [], []
                for c in range(4):
                    qk_ps.append(ps_b.tile([128, 512], f32, name=f"qkps{c}"))
                    vt_ps.append(ps_b.tile([64, 512], f32, name=f"vtps{c}"))
                for e in range(ET):
                    xt_e = xtp.tile([128, SEQ], bf16, name=f"xt{e}",
                                    tag=f"xt{e}")
                    eng = nc.sync if e % 2 == 0 else nc.gpsimd
                    eng.dma_start(out=xt_e[:],
                                  in_=xT[128 * e:128 * (e + 1), :])
                    for c in range(SEQ // 512):
                        nc.tensor.matmul(
                            qk_ps[c][:],
                            lhsT=wqk_sb[:, e, :],
                            rhs=xt_e[:, 512 * c:512 * (c + 1)],
                            start=(e == 0), stop=(e == ET - 1),
                        )
                    for c in range(SEQ // 512):
                        nc.tensor.matmul(
                            vt_ps[c][:],
                            lhsT=wv_sb[:, e, :],
                            rhs=xt_e[:, 512 * c:512 * (c + 1)],
                            start=(e == 0), stop=(e == ET - 1),
                        )
                id_sb = persist.tile([128, 128], f32)
                nc.gpsimd.dma_start(out=id_sb[:], in_=ident[:])

                for c in range(SEQ // 512):
                    nc.vector.tensor_copy(qk_sbs[c][:], qk_ps[c][:])
                    # shift kT (partitions 64:128) down to base partition 0
                    nc.sync.dma_start(out=kt2_sbs[c][0:64, :],
                                      in_=qk_sbs[c][64:128, :])
                    nc.vector.tensor_copy(vt_sbs[c][:], vt_ps[c][:])

            # ---- v layout fix: PE-transpose v^T 128-col slices ----
            with tc.tile_pool(name="ps_vt", bufs=2, space="PSUM") as ps_vt:
                for s in range(ST):
                    v_ps = ps_vt.tile([128, HEAD], bf16, tag="vps")
                    nc.tensor.transpose(
                        v_ps[:],
                        vt_sbs[s // 4][:, 128 * (s % 4):128 * (s % 4 + 1)],
                        id64_sb[:])
                    nc.vector.memset(v_sbs[s][:, HEAD:HEAD + 1], 1.0)
                    nc.vector.memset(v_sbs[s][:, HEAD + 1:128], 0.0)
                    nc.vector.tensor_copy(v_sbs[s][:, 0:HEAD], v_ps[:])

            # ---- Phase D: attention, with per-region finalization ----
            with (
                tc.tile_pool(name="ps_o", bufs=1, space="PSUM") as ps_o,
                tc.tile_pool(name="ps_s", bufs=2, space="PSUM") as ps_s,
            ):
                o_regs = []
                for g in range(4):
                    o_regs.append(ps_o.tile([128, 512], f32,
                                            name=f"oreg{g}", tag=f"oreg{g}"))

                def finalize_region(g):
                    # region g covers queries [512g, 512(g+1))
                    nc.vector.tensor_copy(ot_sb[:, 512 * g:512 * (g + 1)],
                                          o_regs[g][:])
                    for ss in range(4):
                        s = 4 * g + ss
                        t_ps = ps_s.tile([128, 128], f32, tag="sps")
                        nc.tensor.transpose(
                            t_ps[:], ot_sb[:, 128 * s:128 * (s + 1)],
                            id_sb[:])
                        recip = rsb.tile([128, 1], f32, tag="recip")
                        nc.vector.reciprocal(recip[:],
                                             t_ps[:, HEAD:HEAD + 1])
                        o_sb = osb.tile([128, HEAD], f32, tag="osb")
                        nc.vector.tensor_scalar_mul(o_sb[:], t_ps[:, 0:HEAD],
                                                    recip[:])
                        nc.sync.dma_start(
                            out=out[128 * s:128 * (s + 1), :], in_=o_sb[:])

                for j in range(ST):
                    c0 = j // 8
                    lo = 128 * (j % 8)
                    for cc in range(c0, 2):
                        klo = lo if cc == c0 else 0
                        base = 1024 * cc
                        s_ps = ps_s.tile([128, 1024], f32, tag="sps")
                        kT = kt2_sbs[j // 4][:, 128 * (j % 4):128 * (j % 4 + 1)]
                        for h in (0, 512):
                            a = max(klo, h)
                            if a < h + 512:
                                nc.tensor.matmul(
                                    s_ps[:, a:h + 512],
                                    lhsT=kT,
                                    rhs=qk_sbs[2 * cc + h // 512][
                                        :, a - h:a - h + (h + 512 - a)],
                                    start=True, stop=True,
                                )
                        p_sb = psb.tile([128, 1024], bf16, tag="psb")
                        if 0 < klo < 512:
                            nc.vector.memset(p_sb[:, 0:klo], 0.0)
                        elif klo > 512:
                            nc.vector.memset(p_sb[:, 512:klo], 0.0)
                        nc.scalar.activation(p_sb[:, klo:1024],
                                             s_ps[:, klo:1024],
                                             EXP, scale=SCALE)
                        if cc == c0:
                            # causal mask: only the 128-wide diagonal block
                            # needs element masking (upper-tri zeros)
                            nc.vector.tensor_mul(
                                p_sb[:, klo:klo + 128],
                                p_sb[:, klo:klo + 128], mask_sb[:])
                        for hh in (0, 1):
                            g = 2 * cc + hh
                            if j > 4 * g + 3:
                                continue  # fully above causal diagonal
                            nc.tensor.matmul(
                                o_regs[g][:],
                                lhsT=v_sbs[j][:],
                                rhs=p_sb[:, 512 * hh:512 * (hh + 1)],
                                start=(j == 0), stop=(j == 4 * g + 3),
                            )
                            if j == 4 * g + 3:
                                finalize_region(g)

    nc.compile()
    return nc


def _get_program():
    if "nc" not in _CACHE:
        _CACHE["nc"] = _build_program()
    return _CACHE["nc"]


def _host_inputs(x, Wq, Wk, Wv):
    bf16 = ml_dtypes.bfloat16
    # x^T per batch: [E, S] contiguous, bf16
    xT = np.ascontiguousarray(x.transpose(0, 2, 1)).astype(bf16)
    # [Wq | Wk] -> [128, ET, 128] (partition = embed % 128)
    wqk = np.concatenate([Wq, Wk], axis=1).astype(bf16)  # [E, 128]
    wqk = np.ascontiguousarray(
        wqk.reshape(ET, 128, 128).transpose(1, 0, 2))  # [128, ET, 128]
    wv = np.ascontiguousarray(
        Wv.astype(bf16).reshape(ET, 128, HEAD).transpose(1, 0, 2))
    xx = np.arange(128)[:, None]
    yy = np.arange(128)[None, :]
    mask128 = (yy >= xx).astype(bf16)
    ident = np.eye(128, dtype=np.float32)
    ident64 = np.eye(HEAD, dtype=bf16)
    return xT, wqk, wv, mask128, ident, ident64


def kernel(x, Wq, Wk, Wv):
    from concourse.bass_utils import run_bass_kernel_spmd

    nc = _get_program()
    xT, wqk, wv, mask128, ident, ident64 = _host_inputs(x, Wq, Wk, Wv)
    in_maps = [
        {"xT": xT[b], "wqk": wqk, "wv": wv, "mask128": mask128,
         "ident": ident, "ident64": ident64}
        for b in range(BATCH)
    ]
    res = run_bass_kernel_spmd(nc, in_maps, list(range(N_CORES)))
    out = np.stack([np.asarray(res.results[b]["out"]) for b in range(BATCH)])
    return out.astype(np.float32)


# revision 13
# speedup vs baseline: 1.1486x; 1.1033x over previous
"""Trainium2 Bass kernel: single-head causal attention head layer.

Reference computation (per batch b):
    q = x[b] @ Wq; k = x[b] @ Wk; v = x[b] @ Wv        # [S, H], H=64
    w = softmax_causal(q @ k.T * E**-0.5)              # [S, S]
    out[b] = w @ v                                     # [S, H]

Shapes: x (8, 2048, 1024) f32, Wq/Wk/Wv (1024, 64) f32 -> out (8, 2048, 64) f32.

Sharding: data-parallel over batch, one batch per NeuronCore (8 cores).

Device algorithm (per core), all matmuls bf16 with fp32 PSUM accumulation:
  1. Projections pipelined against the x^T DMA stream (per 128-row e-tile):
     [Wq|Wk] stationary -> qk^T psum [128, 2048] (rows 0:64 q^T, 64:128 k^T);
     Wv stationary -> v^T psum [64, 2048].
  2. k^T copied to rows 0:64 of a [128, 2048] tile whose rows 64:128 are
     zero. Scores can then contract over K=128 against the full qk tile
     (zero weights kill the k^T rows): full PE array activity, which keeps
     the HAM clock gate at 2.4 GHz. v^T is PE-transposed into 16 tiles
     v_aug [128, 128]: cols 0:64 = v, col 64 = ones (row sums of P fall out
     of the AV matmul for free), cols 65:128 = zero (pads M to 128).
  3. Scores transposed: S^T[j,i] = k_j . q_i, keys on partitions, so the
     softmax denominator is a partition-dim sum -> folded into step 5.
  4. exp on ScalarE with scale=E**-0.5, 1024-wide chunks. No
     max-subtraction: scaled scores are N(0, 0.0625), exp is safe. Causal
     masking: block skip + multiplicative 0/1 bf16 mask on diagonal chunks.
  5. O^T_aug[h,i] = sum_j v_aug[j,h] P^T[j,i] into four 512-col psum
     region tiles; row 64 = denominators. When a region's last j arrives,
     it is finalized immediately (overlaps the remaining attention):
  6. PE-transpose 128-col slices -> [128, 128]; reciprocal of col 64;
     scale cols 0:64; DMA out fp32.
"""

import numpy as np
import ml_dtypes

BATCH = 8
SEQ = 2048
EMBED = 1024
HEAD = 64
N_CORES = 8
SCALE = float(EMBED) ** -0.5  # 0.03125

ST = SEQ // 128  # 16 seq tiles
ET = EMBED // 128  # 8 embed tiles

_CACHE = {}


def _build_program():
    import concourse.mybir as mybir
    from concourse import bacc
    from concourse.tile import TileContext

    f32 = mybir.dt.float32
    bf16 = mybir.dt.bfloat16
    EXP = mybir.ActivationFunctionType.Exp

    nc = bacc.Bacc("TRN2", target_bir_lowering=False, debug=False,
                   num_devices=N_CORES)

    xT = nc.declare_dram_parameter("xT", [EMBED, SEQ], bf16, isOutput=False)
    wqk = nc.declare_dram_parameter("wqk", [128, ET, 128], bf16, isOutput=False)
    wv = nc.declare_dram_parameter("wv", [128, ET, HEAD], bf16, isOutput=False)
    mask128 = nc.declare_dram_parameter("mask128", [128, 128], bf16,
                                        isOutput=False)
    ident = nc.declare_dram_parameter("ident", [128, 128], f32, isOutput=False)
    ident64 = nc.declare_dram_parameter("ident64", [HEAD, HEAD], bf16,
                                        isOutput=False)
    out = nc.declare_dram_parameter("out", [SEQ, HEAD], f32, isOutput=True)

    with TileContext(nc) as tc:
        with (
            tc.tile_pool(name="persist", bufs=1) as persist,
            tc.tile_pool(name="xtp", bufs=1) as xtp,
            tc.tile_pool(name="vtiles", bufs=1) as vtiles,
            tc.tile_pool(name="psb", bufs=1) as psb,
            tc.tile_pool(name="osb", bufs=4) as osb,
            tc.tile_pool(name="rsb", bufs=4) as rsb,
        ):
            # ---- weights/constants; two issue streams (sync + gpsimd) ----
            wqk_sb = persist.tile([128, ET, 128], bf16)
            nc.sync.dma_start(out=wqk_sb[:], in_=wqk[:])
            wv_sb = persist.tile([128, ET, HEAD], bf16)
            nc.gpsimd.dma_start(out=wv_sb[:], in_=wv[:])
            id64_sb = persist.tile([HEAD, HEAD], bf16)
            nc.gpsimd.dma_start(out=id64_sb[:], in_=ident64[:])
            mask_sb = persist.tile([128, 128], bf16)
            nc.gpsimd.dma_start(out=mask_sb[:], in_=mask128[:])

            # preload the exp table set so ACT_TABLE_LOAD overlaps the DMAs
            warm_sb = persist.tile([128, 1], f32)
            nc.vector.memset(warm_sb[:], 0.0)
            nc.scalar.activation(warm_sb[:], warm_sb[:], EXP, scale=1.0)

            qk_sbs, kt2_sbs, vt_sbs = [], [], []
            for c in range(4):
                qk_sbs.append(persist.tile([128, 512], bf16, name=f"qk{c}"))
                kt2_sbs.append(persist.tile([128, 512], bf16, name=f"kt2{c}"))
                nc.vector.memset(kt2_sbs[c][64:128, :], 0.0)
                vt_sbs.append(persist.tile([64, 512], bf16, name=f"vt{c}"))
            v_sbs = []
            for s in range(ST):
                v_sbs.append(vtiles.tile([128, 128], bf16,
                                         name=f"v{s}", tag=f"v{s}"))
            ot_sb = persist.tile([128, SEQ], f32)

            # ---- Phase B: projections, pipelined against the xT DMA ----
            with tc.tile_pool(name="ps_b", bufs=1, space="PSUM") as ps_b:
                qk_ps, vt_ps = [], []
                for c in range(4):
                    qk_ps.append(ps_b.tile([128, 512], f32, name=f"qkps{c}"))
                    vt_ps.append(ps_b.tile([64, 512], f32, name=f"vtps{c}"))
                for e in range(ET):
                    xt_e = xtp.tile([128, SEQ], bf16, name=f"xt{e}",
                                    tag=f"xt{e}")
                    eng = nc.sync if e % 2 == 0 else nc.gpsimd
                    eng.dma_start(out=xt_e[:],
                                  in_=xT[128 * e:128 * (e + 1), :])
                    for c in range(SEQ // 512):
                        nc.tensor.matmul(
                            qk_ps[c][:],
                            lhsT=wqk_sb[:, e, :],
                            rhs=xt_e[:, 512 * c:512 * (c + 1)],
                            start=(e == 0), stop=(e == ET - 1),
                        )
                    for c in range(SEQ // 512):
                        nc.tensor.matmul(
                            vt_ps[c][:],
                            lhsT=wv_sb[:, e, :],
                            rhs=xt_e[:, 512 * c:512 * (c + 1)],
                            start=(e == 0), stop=(e == ET - 1),
                        )
                id_sb = persist.tile([128, 128], f32)
                nc.gpsimd.dma_start(out=id_sb[:], in_=ident[:])

                for c in range(SEQ // 512):
                    nc.vector.tensor_copy(qk_sbs[c][:], qk_ps[c][:])
                    # shift kT (partitions 64:128) down to base partition 0
                    nc.sync.dma_start(out=kt2_sbs[c][0:64, :],
                                      in_=qk_sbs[c][64:128, :])
                    nc.vector.tensor_copy(vt_sbs[c][:], vt_ps[c][:])

            # ---- v layout fix: PE-transpose v^T 128-col slices ----
            with tc.tile_pool(name="ps_vt", bufs=2, space="PSUM") as ps_vt:
                for s in range(ST):
                    v_ps = ps_vt.tile([128, HEAD], bf16, tag="vps")
                    nc.tensor.transpose(
                        v_ps[:],
                        vt_sbs[s // 4][:, 128 * (s % 4):128 * (s % 4 + 1)],
                        id64_sb[:])
                    nc.vector.memset(v_sbs[s][:, HEAD:HEAD + 1], 1.0)
                    nc.vector.memset(v_sbs[s][:, HEAD + 1:128], 0.0)
                    nc.vector.tensor_copy(v_sbs[s][:, 0:HEAD], v_ps[:])

            # ---- Phase D: attention ----
            # D1: all of P^T = exp(S^T) into SBUF (ACT-paced; PE clock
            # irrelevant). Per completed 512-col region, a dense dep-free
            # O-matmul burst + finalization runs on PE (keeps HAM warm).
            with (
                tc.tile_pool(name="ps_o", bufs=1, space="PSUM") as ps_o,
                tc.tile_pool(name="ps_s", bufs=2, space="PSUM") as ps_s,
            ):
                # PE warmup burst: dep-free matmuls during the
                # projection->attention transition (HAM needs ~3.4us of
                # sustained activity to unthrottle to 2.4 GHz)
                warm_ps = ps_s.tile([128, 512], f32, tag="sps")
                for _ in range(8):
                    nc.tensor.matmul(warm_ps[:], lhsT=wqk_sb[:, 0, :],
                                     rhs=qk_sbs[0][:], start=True, stop=True)

                o_regs = []
                for g in range(4):
                    o_regs.append(ps_o.tile([128, 512], f32,
                                            name=f"oreg{g}", tag=f"oreg{g}"))
                p_tiles = {}

                def finalize_region(g):
                    # dense O burst: all P^T contributions to region g
                    for jj in range(4 * g + 4):
                        hh = g % 2
                        nc.tensor.matmul(
                            o_regs[g][:],
                            lhsT=v_sbs[jj][:],
                            rhs=p_tiles[(jj, g // 2)][
                                :, 512 * hh:512 * (hh + 1)],
                            start=(jj == 0), stop=(jj == 4 * g + 3),
                        )
                    # region g covers queries [512g, 512(g+1))
                    nc.vector.tensor_copy(ot_sb[:, 512 * g:512 * (g + 1)],
                                          o_regs[g][:])
                    for ss in range(4):
                        s = 4 * g + ss
                        t_ps = ps_s.tile([128, 128], f32, tag="sps")
                        nc.tensor.transpose(
                            t_ps[:], ot_sb[:, 128 * s:128 * (s + 1)],
                            id_sb[:])
                        recip = rsb.tile([128, 1], f32, tag="recip")
                        nc.vector.reciprocal(recip[:],
                                             t_ps[:, HEAD:HEAD + 1])
                        o_sb = osb.tile([128, HEAD], f32, tag="osb")
                        nc.vector.tensor_scalar_mul(o_sb[:], t_ps[:, 0:HEAD],
                                                    recip[:])
                        nc.sync.dma_start(
                            out=out[128 * s:128 * (s + 1), :], in_=o_sb[:])

                for j in range(ST):
                    c0 = j // 8
                    lo = 128 * (j % 8)
                    for cc in range(c0, 2):
                        klo = lo if cc == c0 else 0
                        s_ps = ps_s.tile([128, 1024], f32, tag="sps")
                        kT = kt2_sbs[j // 4][:, 128 * (j % 4):128 * (j % 4 + 1)]
                        for h in (0, 512):
                            a = max(klo, h)
                            if a < h + 512:
                                nc.tensor.matmul(
                                    s_ps[:, a:h + 512],
                                    lhsT=kT,
                                    rhs=qk_sbs[2 * cc + h // 512][
                                        :, a - h:a - h + (h + 512 - a)],
                                    start=True, stop=True,
                                )
                        p_sb = psb.tile([128, 1024], bf16,
                                        name=f"p{j}_{cc}", tag=f"p{j}_{cc}")
                        p_tiles[(j, cc)] = p_sb
                        if 0 < klo < 512:
                            nc.vector.memset(p_sb[:, 0:klo], 0.0)
                        elif klo > 512:
                            nc.vector.memset(p_sb[:, 512:klo], 0.0)
                        nc.scalar.activation(p_sb[:, klo:1024],
                                             s_ps[:, klo:1024],
                                             EXP, scale=SCALE)
                        if cc == c0:
                            # causal mask: only the 128-wide diagonal block
                            # needs element masking (upper-tri zeros)
                            nc.vector.tensor_mul(
                                p_sb[:, klo:klo + 128],
                                p_sb[:, klo:klo + 128], mask_sb[:])
                    for g in range(4):
                        if j == 4 * g + 3:
                            finalize_region(g)

    nc.compile()
    return nc


def _get_program():
    if "nc" not in _CACHE:
        _CACHE["nc"] = _build_program()
    return _CACHE["nc"]


def _host_inputs(x, Wq, Wk, Wv):
    bf16 = ml_dtypes.bfloat16
    # x^T per batch: [E, S] contiguous, bf16
    xT = np.ascontiguousarray(x.transpose(0, 2, 1)).astype(bf16)
    # [Wq | Wk] -> [128, ET, 128] (partition = embed % 128)
    wqk = np.concatenate([Wq, Wk], axis=1).astype(bf16)  # [E, 128]
    wqk = np.ascontiguousarray(
        wqk.reshape(ET, 128, 128).transpose(1, 0, 2))  # [128, ET, 128]
    wv = np.ascontiguousarray(
        Wv.astype(bf16).reshape(ET, 128, HEAD).transpose(1, 0, 2))
    xx = np.arange(128)[:, None]
    yy = np.arange(128)[None, :]
    mask128 = (yy >= xx).astype(bf16)
    ident = np.eye(128, dtype=np.float32)
    ident64 = np.eye(HEAD, dtype=bf16)
    return xT, wqk, wv, mask128, ident, ident64


def kernel(x, Wq, Wk, Wv):
    from concourse.bass_utils import run_bass_kernel_spmd

    nc = _get_program()
    xT, wqk, wv, mask128, ident, ident64 = _host_inputs(x, Wq, Wk, Wv)
    in_maps = [
        {"xT": xT[b], "wqk": wqk, "wv": wv, "mask128": mask128,
         "ident": ident, "ident64": ident64}
        for b in range(BATCH)
    ]
    res = run_bass_kernel_spmd(nc, in_maps, list(range(N_CORES)))
    out = np.stack([np.asarray(res.results[b]["out"]) for b in range(BATCH)])
    return out.astype(np.float32)
